# revision 1
# baseline (speedup 1.0000x reference)
"""AttentionAggregation kernel for 8 TRN2 NeuronCores.

Math: out[b] = mean_n softmax(Q K^T)[n,:] @ V  with Q/K/V = x @ W^T + b.
Key algebraic fold: out[b,d] = sum_m w[b,m] V[b,m,d] with
  w[b,m] = (1/N) sum_n exp(S[n,m]) / R[n],  R[n] = sum_m exp(S[n,m]).
So the attn@V matmul (N^2 D work) collapses to a column-weight vector w
computed with rank-1 matmuls (r^T @ E), then a single weighted reduction
against V. Softmax max-subtraction is skipped: |S| <= ~25 here, exp stays
comfortably inside fp32 range and softmax is shift-invariant.

Precision: bf16 inputs/matmuls with fp32 PSUM accumulation end-to-end
rel err ~1.6e-3 (validated numerically against an fp64 reference).

Sharding: core c handles batch b=c//2, row half h=c%2 (2048 rows of the
4096-row softmax). Host sums the two per-core partial outputs per batch.

HW notes learned the hard way:
- fp32 matmuls lower to HI/LO pairs at half stream rate (4x slower than
  bf16 total) -> everything PE-facing is bf16.
- A DVE write (memset) to a PSUM bank that matmuls later accumulate into
  hangs the chip; PSUM zeroing must be done with a matmul (start=True).
- tensor_tensor_reduce faults on HW; use tensor_mul + activation(Identity,
  accum_out=...) instead.
"""

import sys

sys.path.insert(0, "/opt/trn_rl_repo")

import ml_dtypes
import numpy as np

import concourse.bass as bass
import concourse.mybir as mybir
import concourse.tile as tile
from concourse import bacc

D = 128
N = 4096
B = 4
NCORES = 8
HALF = N // 2  # softmax rows per core
RT = HALF // 128  # 16 row tiles per core
GW = 2048  # psum group width (4 banks) per exp instruction
NG = N // GW  # 2 exp groups per row tile
NCH = N // 512  # 8 m-chunks of 512

F32 = mybir.dt.float32
BF16 = mybir.dt.bfloat16
NPBF = ml_dtypes.bfloat16
AF = mybir.ActivationFunctionType
ALU = mybir.AluOpType


def build_nc():
    nc = bacc.Bacc()
    xt = nc.dram_tensor("xt", [D, N], BF16, kind="ExternalInput")  # x[b].T
    xqt = nc.dram_tensor("xqt", [D, HALF], BF16, kind="ExternalInput")  # row-half of x[b].T
    wqT = nc.dram_tensor("wqT", [D, D], BF16, kind="ExternalInput")  # Wq.T
    wkT = nc.dram_tensor("wkT", [D, D], BF16, kind="ExternalInput")
    wvT = nc.dram_tensor("wvT", [D, D], BF16, kind="ExternalInput")
    bq = nc.dram_tensor("bq", [D, 1], F32, kind="ExternalInput")
    bk = nc.dram_tensor("bk", [D, 1], F32, kind="ExternalInput")
    bv = nc.dram_tensor("bv", [D, 1], F32, kind="ExternalInput")
    out = nc.dram_tensor("out", [D, 1], F32, kind="ExternalOutput")

    with tile.TileContext(nc) as tc:
        with (
            tc.tile_pool(name="singles", bufs=1) as singles,
            tc.tile_pool(name="sp", bufs=2, space="PSUM") as sp,
            tc.tile_pool(name="epool", bufs=2) as epool,
        ):
            # ---- constants / weights ----
            wq_sb = singles.tile([D, D], BF16, tag="wq", name="wq_sb")
            wk_sb = singles.tile([D, D], BF16, tag="wk", name="wk_sb")
            wv_sb = singles.tile([D, D], BF16, tag="wv", name="wv_sb")
            bqs = singles.tile([D, 1], F32, tag="bq", name="bqs")
            bks = singles.tile([D, 1], F32, tag="bk", name="bks")
            bvs = singles.tile([D, 1], F32, tag="bv", name="bvs")
            ones_sb = singles.tile([D, D], BF16, tag="ones", name="ones_sb")
            nc.vector.memset(ones_sb, 1.0)

            nc.sync.dma_start(wk_sb, wkT[:, :])
            nc.sync.dma_start(wq_sb, wqT[:, :])
            nc.sync.dma_start(wv_sb, wvT[:, :])

            # ---- activations in (256-col pieces so the ~23GB/s queues overlap) ----
            nc.sync.dma_start(bks, bk[:, :])
            nc.sync.dma_start(bqs, bq[:, :])
            nc.sync.dma_start(bvs, bv[:, :])
            xq_sb = singles.tile([D, HALF], BF16, tag="xq", name="xq_sb")
            xt_sb = singles.tile([D, N], BF16, tag="xt", name="xt_sb")
            for c in range(4):
                nc.sync.dma_start(xq_sb[:, c * 256 : (c + 1) * 256], xqt[:, c * 256 : (c + 1) * 256])
            for c in range(N // 256):
                nc.sync.dma_start(xt_sb[:, c * 256 : (c + 1) * 256], xt[:, c * 256 : (c + 1) * 256])
            for c in range(4, HALF // 256):
                nc.sync.dma_start(xq_sb[:, c * 256 : (c + 1) * 256], xqt[:, c * 256 : (c + 1) * 256])

            kt_sb = singles.tile([D, N], BF16, tag="kt", name="kt_sb")
            qt_sb = singles.tile([D, HALF], BF16, tag="qt", name="qt_sb")
            vt_sb = singles.tile([D, N], F32, tag="vt", name="vt_sb")

            # ---- projections (bf16 matmul, fp32 psum, bias add on copyback) ----
            def proj_group(dst, w_sb, src_sb, bias_sb, g, pfx, on_act=False):
                pt = sp.tile([128, GW], F32, tag="spg", name=f"{pfx}_{g}")
                for hh in range(2):
                    s0 = g * 1024 + hh * 512
                    nc.tensor.matmul(
                        pt[:, hh * 512 : (hh + 1) * 512],
                        w_sb,
                        src_sb[:, s0 : s0 + 512],
                        start=True,
                        stop=True,
                    )
                if on_act:
                    # ACT is idle before the first exp; bias rides the free affine
                    nc.scalar.activation(
                        out=dst[:, g * 1024 : (g + 1) * 1024],
                        in_=pt[:, 0:1024],
                        func=AF.Identity,
                        bias=bias_sb,
                    )
                else:
                    nc.vector.tensor_scalar_add(
                        out=dst[:, g * 1024 : (g + 1) * 1024], in0=pt[:, 0:1024], scalar1=bias_sb
                    )

            def proj(dst, w_sb, src_sb, bias_sb, width, pfx, on_act=False):
                for g in range(width // 1024):
                    proj_group(dst, w_sb, src_sb, bias_sb, g, pfx, on_act=(on_act and g % 2 == 0))

            proj(kt_sb, wk_sb, xt_sb, bks, N, "kp", on_act=True)
            proj(qt_sb, wq_sb, xq_sb, bqs, HALF, "qp", on_act=True)

            # ---- per-tile scalar arrays (no pool rotation -> no slot waits) ----
            part_all = singles.tile([128, RT * NG], F32, tag="part", name="part_all")
            R_all = singles.tile([128, RT], F32, tag="R", name="R_all")
            rr_all = singles.tile([128, RT], F32, tag="rr", name="rr_all")
            # rmat[jj] is a [128,128] stationary with r in column 32*jj and
            # zeros elsewhere: r^T@E lands on partition 32*jj, exact zeros on
            # the rest, so accumulating chunks in one bank never contaminates
            # and every PSUM partition is matmul-written (keeps CoreSim happy).
            rmat = singles.tile([128, 4, D], BF16, tag="rmat", name="rmat")
            nc.vector.memset(rmat, 0.0)
            # fp32 SBUF accumulator for w; chunk j lives at partition 32*(j%4),
            # columns (j//4)*512.., matching the transient psum layout.
            wacc = singles.tile([128, 1024], F32, tag="wacc", name="wacc")
            nc.vector.memset(wacc, 0.0)

            def emit_S(i):
                lhsT = qt_sb[:, i * 128 : (i + 1) * 128]
                tiles = []
                for g in range(NG):
                    t = sp.tile([128, GW], F32, tag="spg", name=f"sp_{i}_{g}")
                    for hh in range(GW // 512):
                        m0 = g * GW + hh * 512
                        nc.tensor.matmul(
                            t[:, hh * 512 : (hh + 1) * 512],
                            lhsT,
                            kt_sb[:, m0 : m0 + 512],
                            start=True,
                            stop=True,
                        )
                    tiles.append(t)
                return tiles

            def emit_exp(i, sptiles):
                E = epool.tile([128, N], BF16, tag="E", name=f"E_{i}")
                for g in range(NG):
                    nc.scalar.activation(
                        out=E[:, g * GW : (g + 1) * GW],
                        in_=sptiles[g],
                        func=AF.Exp,
                        accum_out=part_all[:, NG * i + g : NG * i + g + 1],
                    )
                return E

            def emit_r(i):
                nc.vector.tensor_reduce(
                    out=R_all[:, i : i + 1],
                    in_=part_all[:, NG * i : NG * (i + 1)],
                    axis=mybir.AxisListType.X,
                    op=ALU.add,
                )
                nc.vector.reciprocal(out=rr_all[:, i : i + 1], in_=R_all[:, i : i + 1])
                for jj in range(4):
                    nc.vector.tensor_copy(
                        out=rmat[:, jj, 32 * jj : 32 * jj + 1], in_=rr_all[:, i : i + 1]
                    )

            def emit_w(i, E):
                # transient psum: [128,1024] = 2 banks; chunk j contributes
                # r*E_j on partition 32*(j%4) of bank j//4 via its rmat.
                wt = sp.tile([128, 1024], F32, tag="spg", name=f"wt_{i}")
                for j in range(NCH):
                    jj = j % 4
                    half = j // 4
                    nc.tensor.matmul(
                        wt[:, half * 512 : (half + 1) * 512],
                        rmat[:, jj, :],
                        E[:, j * 512 : (j + 1) * 512],
                        start=(jj == 0),
                        stop=(jj == 3),
                        skip_group_check=True,
                    )
                # accumulate into SBUF (frees the psum slot for the next S tile)
                for half in range(2):
                    nc.vector.tensor_add(
                        out=wacc[:, half * 512 : (half + 1) * 512],
                        in0=wacc[:, half * 512 : (half + 1) * 512],
                        in1=wt[:, half * 512 : (half + 1) * 512],
                    )

            # ---- main loop, software-pipelined emission ----
            exps = {}
            exps[0] = emit_exp(0, emit_S(0))
            exps[1] = emit_exp(1, emit_S(1))
            for i in range(RT):
                E = exps.pop(i)
                emit_r(i)
                emit_w(i, E)
                # V projection is off the critical path; one group per early
                # iteration spreads its PSUM slot pressure.
                if 2 <= i < 2 + N // 1024:
                    proj_group(vt_sb, wv_sb, xt_sb, bvs, i - 2, "vp")
                if i + 2 < RT:
                    exps[i + 2] = emit_exp(i + 2, emit_S(i + 2))

            # ---- epilogue: replicate w across partitions, contract with V^T ----
            wbb = singles.tile([128, 1024], BF16, tag="wbb", name="wbb")
            opart = singles.tile([128, NCH], F32, tag="opart", name="opart")
            nc.vector.tensor_copy(out=wbb, in_=wacc)
            for j in range(NCH):
                jj = j % 4
                half = j // 4
                cs = slice(half * 512, (half + 1) * 512)
                wrep = sp.tile([128, 512], F32, tag="spg", name=f"wrep_{j}")
                nc.tensor.matmul(
                    wrep[:, 0:512],
                    ones_sb[32 * jj : 32 * jj + 1, :],
                    wbb[32 * jj : 32 * jj + 1, cs],
                    start=True,
                    stop=True,
                    tile_position=(32 * jj, 0),
                )
                scratch = epool.tile([128, 512], F32, tag="scr", name=f"scr_{j}")
                scratch2 = epool.tile([128, 512], F32, tag="scr2", name=f"scr2_{j}")
                nc.vector.tensor_mul(
                    out=scratch, in0=vt_sb[:, j * 512 : (j + 1) * 512], in1=wrep[:, 0:512]
                )
                # ACT does the free-dim sum (accum_out) while DVE moves on
                nc.scalar.activation(
                    out=scratch2,
                    in_=scratch,
                    func=AF.Identity,
                    scale=1.0 / N,
                    accum_out=opart[:, j : j + 1],
                )
            o128 = singles.tile([128, 1], F32, tag="o128", name="o128")
            nc.vector.tensor_reduce(out=o128, in_=opart, axis=mybir.AxisListType.X, op=ALU.add)
            nc.sync.dma_start(out[:, :], o128)

    nc.compile()
    return nc


_cache = {}


def get_nc():
    if "nc" not in _cache:
        _cache["nc"] = build_nc()
    return _cache["nc"]


def make_in_maps(x, Wq, bq, Wk, bk, Wv, bv):
    x = np.asarray(x, np.float32)
    wqT = np.ascontiguousarray(np.asarray(Wq, np.float32).T.astype(NPBF))
    wkT = np.ascontiguousarray(np.asarray(Wk, np.float32).T.astype(NPBF))
    wvT = np.ascontiguousarray(np.asarray(Wv, np.float32).T.astype(NPBF))
    bqc = np.ascontiguousarray(np.asarray(bq, np.float32).reshape(D, 1))
    bkc = np.ascontiguousarray(np.asarray(bk, np.float32).reshape(D, 1))
    bvc = np.ascontiguousarray(np.asarray(bv, np.float32).reshape(D, 1))
    in_maps = []
    for c in range(NCORES):
        b = c // 2
        h = c % 2
        xbT = np.ascontiguousarray(x[b].T.astype(NPBF))  # [128, 4096] bf16
        in_maps.append(
            {
                "xt": xbT,
                "xqt": np.ascontiguousarray(xbT[:, h * HALF : (h + 1) * HALF]),
                "wqT": wqT,
                "wkT": wkT,
                "wvT": wvT,
                "bq": bqc,
                "bk": bkc,
                "bv": bvc,
            }
        )
    return in_maps


def combine(results):
    outs = [np.asarray(results[c]["out"]).reshape(D) for c in range(NCORES)]
    return np.stack([outs[2 * b] + outs[2 * b + 1] for b in range(B)]).astype(np.float32)


def run(inputs, trace=False, **kwargs):
    from concourse.bass_utils import run_bass_kernel_spmd

    nc = get_nc()
    in_maps = make_in_maps(**inputs)
    res = run_bass_kernel_spmd(nc, in_maps, core_ids=list(range(NCORES)), trace=trace, **kwargs)
    return combine(res.results), res


def kernel(x, Wq, bq, Wk, bk, Wv, bv):
    out, _ = run(dict(x=x, Wq=Wq, bq=bq, Wk=Wk, bk=bk, Wv=Wv, bv=bv))
    return out



# revision 6
# speedup vs baseline: 1.0109x; 1.0109x over previous
"""AttentionAggregation kernel for 8 TRN2 NeuronCores (v2: ACT-bound pipeline).

Math: out[b] = mean_n softmax(Q K^T)[n,:] @ V  with Q/K/V = x @ W^T + b.
Fold: out[b,d] = sum_m w[b,m] V[b,m,d],  w[b,m] = (1/N) sum_n exp(S[n,m])/R[n],
R[n] = sum_m exp(S[n,m]).  attn@V collapses to rank-1 matmuls (r^T @ E) plus a
single weighted reduction against V.  Softmax max-subtraction skipped (|S|<~25).

Sharding: core c -> batch b=c//2, softmax-row half h=c%2 (2048 rows each).
Host permutes x[b].T columns so each core's own row-half comes first; the m
(key) axis is consistently permuted for K/V so softmax/out are unaffected.
Host sums the two per-core partials per batch.

v2 pipeline (per core): ACT is the hard floor (exp of 2048x4096 elems at
1 elem/lane/cycle @1.2GHz => ~4.57us per 128-row tile).  PSUM is split into
two persistent [128,2048] halves L=banks0-3 / R=banks4-7; the S matmuls for
tile i+1 and the w rank-1 matmuls for tile i time-share the banks in the
gaps between exp reads, so PE (~3.4us/tile) hides completely under ACT.
w accumulates per tile into bank 0/4 (4 chunks at partition offsets 32j),
drained by DVE into an SBUF accumulator.  PE HAM warm-up dummies and an
early exp (activation-table load, ~2.7us) run during the input DMA wait.

HW notes (inherited + new):
- fp32 matmuls lower to HI/LO pairs -> everything PE-facing is bf16.
- DVE memset of a PSUM bank that matmuls later accumulate into hangs the
  chip; PSUM zeroing only via matmul start=True.
- tensor_tensor_reduce faults on HW; use tensor ops + separate reduce.
- ACT ACTIVATE costs (N+352)/1.2 ns; accum_out adds a 284ns READ_ACC instr.
"""

import sys

sys.path.insert(0, "/opt/trn_rl_repo")

import ml_dtypes
import numpy as np

import concourse.bass as bass
import concourse.mybir as mybir
import concourse.tile as tile
from concourse import bacc

D = 128
N = 4096
B = 4
NCORES = 8
HALF = N // 2  # softmax rows per core
RT = HALF // 128  # 16 row tiles per core

F32 = mybir.dt.float32
BF16 = mybir.dt.bfloat16
NPBF = ml_dtypes.bfloat16
AF = mybir.ActivationFunctionType
ALU = mybir.AluOpType


def build_nc():
    nc = bacc.Bacc()
    xt = nc.dram_tensor("xt", [D, N], BF16, kind="ExternalInput")  # x[b].T, q-half first
    wqT = nc.dram_tensor("wqT", [D, D], BF16, kind="ExternalInput")  # Wq.T
    wkT = nc.dram_tensor("wkT", [D, D], BF16, kind="ExternalInput")
    wvT = nc.dram_tensor("wvT", [D, D], BF16, kind="ExternalInput")
    bq = nc.dram_tensor("bq", [D, 1], F32, kind="ExternalInput")
    bk = nc.dram_tensor("bk", [D, 1], F32, kind="ExternalInput")
    bv = nc.dram_tensor("bv", [D, 1], F32, kind="ExternalInput")
    out = nc.dram_tensor("out", [D, 1], F32, kind="ExternalOutput")

    with tile.TileContext(nc) as tc:
        with (
            tc.tile_pool(name="singles", bufs=1) as singles,
            tc.tile_pool(name="pp", bufs=1, space="PSUM") as pp,
            tc.tile_pool(name="epool", bufs=3) as epool,
        ):
            # ---- persistent PSUM halves: L = banks 0-3, R = banks 4-7 ----
            L = pp.tile([128, 2048], F32, tag="L", name="L")
            R = pp.tile([128, 2048], F32, tag="R", name="R")

            # ---- SBUF singles ----
            wq_sb = singles.tile([D, D], BF16, tag="wq", name="wq_sb")
            wk_sb = singles.tile([D, D], BF16, tag="wk", name="wk_sb")
            wv_sb = singles.tile([D, D], BF16, tag="wv", name="wv_sb")
            bqs = singles.tile([D, 1], F32, tag="bq", name="bqs")
            bks = singles.tile([D, 1], F32, tag="bk", name="bks")
            bvs = singles.tile([D, 1], F32, tag="bv", name="bvs")
            ones_sb = singles.tile([D, D], BF16, tag="ones", name="ones_sb")
            tl_out = singles.tile([D, 1], F32, tag="tl", name="tl_out")
            xt_sb = singles.tile([D, N], BF16, tag="xt", name="xt_sb")
            kt_sb = singles.tile([D, N], BF16, tag="kt", name="kt_sb")
            qt_sb = singles.tile([D, HALF], BF16, tag="qt", name="qt_sb")
            vt_sb = singles.tile([D, N], F32, tag="vt", name="vt_sb")
            part = singles.tile([128, 2 * RT], F32, tag="part", name="part")
            Rcol = singles.tile([128, RT], F32, tag="R", name="Rcol")
            rr = singles.tile([128, RT], F32, tag="rr", name="rr")
            # rmat[:, p, j, 32j] = rr for tile parity p; zeros elsewhere so the
            # 4-chunk psum accumulation stays exact on every partition.
            rmat = singles.tile([128, 2, 4, D], BF16, tag="rmat", name="rmat")
            wacc = singles.tile([128, 1024], F32, tag="wacc", name="wacc")
            wbb = singles.tile([128, 1024], BF16, tag="wbb", name="wbb")
            escr = singles.tile([128, 2048], F32, tag="escr", name="escr")
            odump = singles.tile([128, 1024], F32, tag="odump", name="odump")
            opart = singles.tile([128, 4], F32, tag="opart", name="opart")
            o1 = singles.tile([128, 1], F32, tag="o1", name="o1")
            o128 = singles.tile([128, 1], F32, tag="o128", name="o128")

            nc.vector.memset(ones_sb, 1.0)
            nc.vector.memset(rmat, 0.0)
            nc.vector.memset(wacc, 0.0)

            # ---- DMAs (xt in 256-col pieces across queues; weights first) ----
            nc.sync.dma_start(wk_sb, wkT[:, :])
            nc.sync.dma_start(wq_sb, wqT[:, :])
            nc.sync.dma_start(bks, bk[:, :])
            nc.sync.dma_start(bqs, bq[:, :])
            for c in range(8):
                nc.sync.dma_start(xt_sb[:, c * 256 : (c + 1) * 256], xt[:, c * 256 : (c + 1) * 256])
            nc.sync.dma_start(wv_sb, wvT[:, :])
            nc.sync.dma_start(bvs, bv[:, :])
            for c in range(8, 16):
                nc.sync.dma_start(xt_sb[:, c * 256 : (c + 1) * 256], xt[:, c * 256 : (c + 1) * 256])

            # ---- ACT: trigger exp table-load early (costs ~2.7us once) ----
            nc.scalar.activation(out=tl_out, in_=ones_sb[:, 0:1], func=AF.Exp)

            # ---- PE HAM warm-up while DMA lands (keeps clock at 2.4GHz) ----
            for _ in range(20):
                nc.tensor.matmul(
                    R[:, 1920:2048], ones_sb, ones_sb, start=True, stop=True
                )

            # ---- projections ----
            # K left half -> L banks, drain (bias+cast) to kt_sb
            for g in range(4):
                nc.tensor.matmul(
                    L[:, g * 512 : (g + 1) * 512],
                    wk_sb,
                    xt_sb[:, g * 512 : (g + 1) * 512],
                    start=True,
                    stop=True,
                )
            # Q (this core's row half = xt cols 0..2047) -> R banks
            for g in range(4):
                nc.tensor.matmul(
                    R[:, g * 512 : (g + 1) * 512],
                    wq_sb,
                    xt_sb[:, g * 512 : (g + 1) * 512],
                    start=True,
                    stop=True,
                )
            for g in range(2):
                nc.vector.tensor_scalar_add(
                    out=kt_sb[:, g * 1024 : (g + 1) * 1024],
                    in0=L[:, g * 1024 : (g + 1) * 1024],
                    scalar1=bks,
                )
            # qt tile 0 first so S_L(0) can start ASAP
            nc.vector.tensor_scalar_add(out=qt_sb[:, 0:128], in0=R[:, 0:128], scalar1=bqs)
            nc.vector.tensor_scalar_add(out=qt_sb[:, 128:1024], in0=R[:, 128:1024], scalar1=bqs)
            nc.vector.tensor_scalar_add(out=qt_sb[:, 1024:2048], in0=R[:, 1024:2048], scalar1=bqs)

            E_tiles = {}

            def emit_S(i, half):
                """Stage S for row tile i into L (half=0) or R (half=1).
                Emits cols 512..2048 first (banks 1-3), then cols 0..512
                (bank 0/4, which the w matmuls + drain may still occupy)."""
                reg = L if half == 0 else R
                lhsT = qt_sb[:, i * 128 : (i + 1) * 128]
                for g in (1, 2, 3, 0):
                    m0 = half * 2048 + g * 512
                    nc.tensor.matmul(
                        reg[:, g * 512 : (g + 1) * 512],
                        lhsT,
                        kt_sb[:, m0 : m0 + 512],
                        start=True,
                        stop=True,
                    )

            def emit_exp(i, half):
                if i not in E_tiles:
                    E_tiles[i] = epool.tile([128, N], BF16, tag="E", name=f"E_{i}")
                reg = L if half == 0 else R
                nc.scalar.activation(
                    out=E_tiles[i][:, half * 2048 : (half + 1) * 2048],
                    in_=reg,
                    func=AF.Exp,
                    accum_out=part[:, 2 * i + half : 2 * i + half + 1],
                )

            def emit_rr(i):
                nc.vector.tensor_add(
                    out=Rcol[:, i : i + 1],
                    in0=part[:, 2 * i : 2 * i + 1],
                    in1=part[:, 2 * i + 1 : 2 * i + 2],
                )
                nc.vector.reciprocal(out=rr[:, i : i + 1], in_=Rcol[:, i : i + 1])
                p = i % 2
                for j in range(4):
                    nc.vector.tensor_copy(
                        out=rmat[:, p, j, 32 * j : 32 * j + 1], in_=rr[:, i : i + 1]
                    )

            def emit_w(i, half):
                """Rank-1 contraction of E tile i, m-half `half`, into bank 0
                (L) or bank 4 (R): chunk j lands on partition 32j."""
                reg = L if half == 0 else R
                p = i % 2
                E = E_tiles[i]
                for j in range(4):
                    m0 = half * 2048 + j * 512
                    nc.tensor.matmul(
                        reg[:, 0:512],
                        rmat[:, p, j, :],
                        E[:, m0 : m0 + 512],
                        start=(j == 0),
                        stop=(j == 3),
                        skip_group_check=True,
                    )

            def emit_wdrain(i, half):
                reg = L if half == 0 else R
                nc.vector.tensor_add(
                    out=wacc[:, half * 512 : (half + 1) * 512],
                    in0=wacc[:, half * 512 : (half + 1) * 512],
                    in1=reg[:, 0:512],
                )

            def emit_vproj(c):
                """V projection chunk c (1024 cols) through L banks 1-2 in the
                gap after exp reads them; drain (bias, keep fp32) to vt_sb."""
                for g in range(2):
                    nc.tensor.matmul(
                        L[:, 512 + g * 512 : 512 + (g + 1) * 512],
                        wv_sb,
                        xt_sb[:, c * 1024 + g * 512 : c * 1024 + (g + 1) * 512],
                        start=True,
                        stop=True,
                    )
                nc.vector.tensor_scalar_add(
                    out=vt_sb[:, c * 1024 : (c + 1) * 1024], in0=L[:, 512:1536], scalar1=bvs
                )

            # ---- prologue: first tile ----
            emit_S(0, 0)
            emit_exp(0, 0)
            # K right half through R banks (Q psum already drained), during expL(0)
            for g in range(4):
                nc.tensor.matmul(
                    R[:, g * 512 : (g + 1) * 512],
                    wk_sb,
                    xt_sb[:, 2048 + g * 512 : 2048 + (g + 1) * 512],
                    start=True,
                    stop=True,
                )
            for g in range(2):
                nc.vector.tensor_scalar_add(
                    out=kt_sb[:, 2048 + g * 1024 : 2048 + (g + 1) * 1024],
                    in0=R[:, g * 1024 : (g + 1) * 1024],
                    scalar1=bks,
                )
            emit_S(0, 1)
            emit_exp(0, 1)

            # ---- main loop ----
            # Steady state (one exp instr = one 2.28us "window"):
            #   during exp(i,1):   PE does w(i-1,0), vproj, S(i+1,0)
            #   during exp(i+1,0): PE does w(i-1,1), S(i+1,1)
            # so PE (~1.7-2.1us/window) hides fully under ACT and the banks
            # hand over L0/R0 in the gaps between exp reads.
            for i in range(RT):
                emit_rr(i)
                if i + 1 < RT:
                    emit_S(i + 1, 0)
                    emit_exp(i + 1, 0)
                if i >= 1:
                    emit_w(i - 1, 1)
                    emit_wdrain(i - 1, 1)
                if i + 1 < RT:
                    emit_S(i + 1, 1)
                    emit_exp(i + 1, 1)
                emit_w(i, 0)
                emit_wdrain(i, 0)
                if i < 4:
                    emit_vproj(i)
            emit_w(RT - 1, 1)
            emit_wdrain(RT - 1, 1)

            # ---- epilogue: out[d] = (1/N) sum_m w[m] vt[d, m] ----
            nc.vector.tensor_copy(out=wbb, in_=wacc)
            for pr in range(4):  # 1024-col m chunks
                hf, j = pr // 2, (pr % 2) * 2
                reg = L if pr % 2 == 0 else R
                for jj in (j, j + 1):
                    # replicate w segment (partition 32jj of wbb) to all partitions
                    nc.tensor.matmul(
                        reg[:, (jj - j) * 512 : (jj - j + 1) * 512],
                        ones_sb[32 * jj : 32 * jj + 1, :],
                        wbb[32 * jj : 32 * jj + 1, hf * 512 : (hf + 1) * 512],
                        start=True,
                        stop=True,
                        tile_position=(32 * jj, 0),
                    )
                m0 = hf * 2048 + j * 512
                scr = escr[:, (pr % 2) * 1024 : (pr % 2 + 1) * 1024]
                nc.vector.tensor_mul(out=scr, in0=vt_sb[:, m0 : m0 + 1024], in1=reg[:, 0:1024])
                if pr % 2 == 0:
                    # ACT is idle after the last exp; use its free-dim accumulator
                    nc.scalar.activation(
                        out=odump,
                        in_=scr,
                        func=AF.Identity,
                        accum_out=opart[:, pr : pr + 1],
                    )
                else:
                    nc.vector.tensor_reduce(
                        out=opart[:, pr : pr + 1], in_=scr, axis=mybir.AxisListType.X, op=ALU.add
                    )
            nc.vector.tensor_reduce(out=o1, in_=opart, axis=mybir.AxisListType.X, op=ALU.add)
            nc.scalar.activation(out=o128, in_=o1, func=AF.Identity, scale=1.0 / N)
            nc.sync.dma_start(out[:, :], o128)

    nc.compile()
    return nc


_cache = {}


def get_nc():
    if "nc" not in _cache:
        _cache["nc"] = build_nc()
    return _cache["nc"]


def make_in_maps(x, Wq, bq, Wk, bk, Wv, bv):
    x = np.asarray(x, np.float32)
    wqT = np.ascontiguousarray(np.asarray(Wq, np.float32).T.astype(NPBF))
    wkT = np.ascontiguousarray(np.asarray(Wk, np.float32).T.astype(NPBF))
    wvT = np.ascontiguousarray(np.asarray(Wv, np.float32).T.astype(NPBF))
    bqc = np.ascontiguousarray(np.asarray(bq, np.float32).reshape(D, 1))
    bkc = np.ascontiguousarray(np.asarray(bk, np.float32).reshape(D, 1))
    bvc = np.ascontiguousarray(np.asarray(bv, np.float32).reshape(D, 1))
    in_maps = []
    for c in range(NCORES):
        b = c // 2
        h = c % 2
        xbT = x[b].T.astype(NPBF)  # [128, 4096] bf16
        # this core's own row-half first (m axis consistently permuted)
        xperm = np.ascontiguousarray(
            np.concatenate([xbT[:, h * HALF : (h + 1) * HALF], xbT[:, (1 - h) * HALF : (2 - h) * HALF]], axis=1)
        )
        in_maps.append(
            {
                "xt": xperm,
                "wqT": wqT,
                "wkT": wkT,
                "wvT": wvT,
                "bq": bqc,
                "bk": bkc,
                "bv": bvc,
            }
        )
    return in_maps


def combine(results):
    outs = [np.asarray(results[c]["out"]).reshape(D) for c in range(NCORES)]
    return np.stack([outs[2 * b] + outs[2 * b + 1] for b in range(B)]).astype(np.float32)


def run(inputs, trace=False, **kwargs):
    from concourse.bass_utils import run_bass_kernel_spmd

    nc = get_nc()
    in_maps = make_in_maps(**inputs)
    res = run_bass_kernel_spmd(nc, in_maps, core_ids=list(range(NCORES)), trace=trace, **kwargs)
    return combine(res.results), res


def kernel(x, Wq, bq, Wk, bk, Wv, bv):
    out, _ = run(dict(x=x, Wq=Wq, bq=bq, Wk=Wk, bk=bk, Wv=Wv, bv=bv))
    return out


# revision 10
# speedup vs baseline: 1.0433x; 1.0321x over previous
"""AttentionAggregation kernel for 8 TRN2 NeuronCores (v3).

Math: out[b] = mean_n softmax(Q K^T)[n,:] @ V  with Q/K/V = x @ W^T + b.
Fold: out[b,d] = sum_m w[b,m] V[b,m,d],  w[b,m] = (1/N) sum_n exp(S[n,m])/R[n],
R[n] = sum_m exp(S[n,m]).  attn@V collapses to rank-1 matmuls (r^T @ E) plus a
single weighted reduction against V.  Softmax max-subtraction skipped (|S|<~25).

Sharding: core c -> batch b=c//2, softmax-row half h=c%2 (2048 rows each).
Host permutes x[b].T columns so each core's own row-half comes first (the m
axis is consistently permuted for K/V; softmax and the final sum are
permutation-invariant).  Host sums the two per-core partials and adds bv
(exact: each core's sum_m w[m] = 0.5).

v3 pipeline notes (trace-driven):
- ACT is the pacing engine: one 2048-wide exp = (2048+352)/1.2 = 2.0us, two
  per 128-row tile + 2 READ_ACC = 4.57us/tile floor.
- PSUM = two persistent [128,2048] tiles L (banks 0-3) / R (banks 4-7).
  Tile-framework deps are PSUM-tile-granular, so everything touching L
  serializes: exp_L(i) -> w(i-1,0) -> cast -> S(i+1,0) -> exp_L(i+1).
  v3 shortens that chain: the w psum is CAST (not read-modify-added) to a
  bf16 slot on DVE, and the accumulation into wacc happens on the otherwise
  idle GPSIMD engine, off the chain.  rmat copies also live on GPSIMD.
- V projection is fused into the epilogue (V chunks computed from xt right
  before the final multiply), so the main loop/prologue carry no V work.
- x arrives as 8 contiguous [128,512] pieces (1KB DMA elements instead of
  512B strided rows) issued from sync+scalar+gpsimd queues in parallel.
- PE HAM warm-up dummies + early exp-table load run during the DMA wait.

HW notes (inherited):
- everything PE-facing is bf16 (fp32 matmuls lower to HI/LO pairs).
- no DVE/ACT writes to PSUM banks that matmuls later accumulate into.
- tensor_tensor_reduce faults on HW; keep mult and reduce separate.
"""

import sys

sys.path.insert(0, "/opt/trn_rl_repo")

import ml_dtypes
import numpy as np

import concourse.bass as bass
import concourse.mybir as mybir
import concourse.tile as tile
from concourse import bacc

D = 128
N = 4096
B = 4
NCORES = 8
HALF = N // 2
RT = HALF // 128  # 16 row tiles per core

F32 = mybir.dt.float32
BF16 = mybir.dt.bfloat16
NPBF = ml_dtypes.bfloat16
AF = mybir.ActivationFunctionType
ALU = mybir.AluOpType


def build_nc():
    nc = bacc.Bacc()
    xt = nc.dram_tensor("xt", [8, D, 512], BF16, kind="ExternalInput")  # x[b].T pieces
    wqT = nc.dram_tensor("wqT", [D, D], BF16, kind="ExternalInput")
    wkT = nc.dram_tensor("wkT", [D, D], BF16, kind="ExternalInput")
    wvT = nc.dram_tensor("wvT", [D, D], BF16, kind="ExternalInput")
    bq = nc.dram_tensor("bq", [D, 1], F32, kind="ExternalInput")
    bk = nc.dram_tensor("bk", [D, 1], F32, kind="ExternalInput")
    out = nc.dram_tensor("out", [D, 1], F32, kind="ExternalOutput")

    with tile.TileContext(nc) as tc:
        with (
            tc.tile_pool(name="singles", bufs=1) as singles,
            tc.tile_pool(name="pp", bufs=1, space="PSUM") as pp,
            tc.tile_pool(name="epool", bufs=3) as epool,
        ):
            L = pp.tile([128, 2048], F32, tag="L", name="L")
            R = pp.tile([128, 2048], F32, tag="R", name="R")

            wq_sb = singles.tile([D, D], BF16, tag="wq", name="wq_sb")
            wk_sb = singles.tile([D, D], BF16, tag="wk", name="wk_sb")
            wv_sb = singles.tile([D, D], BF16, tag="wv", name="wv_sb")
            bqs = singles.tile([D, 1], F32, tag="bq", name="bqs")
            bks = singles.tile([D, 1], F32, tag="bk", name="bks")
            ones_sb = singles.tile([D, D], BF16, tag="ones", name="ones_sb")
            tl_out = singles.tile([D, 1], F32, tag="tl", name="tl_out")
            xt_sb = singles.tile([D, N], BF16, tag="xt", name="xt_sb")
            kt_sb = singles.tile([D, N], BF16, tag="kt", name="kt_sb")
            qt_sb = singles.tile([D, HALF], BF16, tag="qt", name="qt_sb")
            part = singles.tile([128, 2 * RT], F32, tag="part", name="part")
            Rcol = singles.tile([128, RT], F32, tag="R", name="Rcol")
            rr = singles.tile([128, RT], F32, tag="rr", name="rr")
            rmat = singles.tile([128, 2, 4, D], BF16, tag="rmat", name="rmat")
            # per-tile w slots (bf16) cast off PSUM; GPSIMD folds them into wacc
            wstore = singles.tile([128, 4, 512], BF16, tag="wst", name="wstore")
            wacc = singles.tile([128, 1024], BF16, tag="wacc", name="wacc")
            escr = singles.tile([128, 2048], F32, tag="escr", name="escr")
            vsb = singles.tile([128, 2048], BF16, tag="vsb", name="vsb")
            odump = singles.tile([128, 1024], F32, tag="odump", name="odump")
            opart = singles.tile([128, 4], F32, tag="opart", name="opart")
            o1 = singles.tile([128, 1], F32, tag="o1", name="o1")
            o128 = singles.tile([128, 1], F32, tag="o128", name="o128")

            nc.vector.memset(ones_sb, 1.0)
            nc.vector.memset(rmat, 0.0)
            nc.gpsimd.memset(wacc, 0.0)

            # ---- DMAs: parallel issue across sync / scalar / gpsimd ----
            nc.sync.dma_start(wk_sb, wkT[:, :])
            for c in range(4):
                nc.sync.dma_start(xt_sb[:, c * 512 : (c + 1) * 512], xt[c, :, :])
            nc.scalar.dma_start(wq_sb, wqT[:, :])
            nc.scalar.dma_start(bks, bk[:, :])
            nc.scalar.dma_start(bqs, bq[:, :])
            nc.scalar.dma_start(wv_sb, wvT[:, :])
            for c in range(4, 8):
                nc.gpsimd.dma_start(xt_sb[:, c * 512 : (c + 1) * 512], xt[c, :, :])

            # early exp table load (~2.7us) while DMAs land
            nc.scalar.activation(out=tl_out, in_=ones_sb[:, 0:1], func=AF.Exp)

            # PE HAM warm-up (keeps clock at 2.4GHz through the prologue)
            for _ in range(20):
                nc.tensor.matmul(R[:, 1024:1152], ones_sb, ones_sb, start=True, stop=True)

            # ---- projections (1024-wide matmuls, bias-add drains on DVE) ----
            for g in range(4):  # K left -> L
                nc.tensor.matmul(
                    L[:, g * 512 : (g + 1) * 512],
                    wk_sb,
                    xt_sb[:, g * 512 : (g + 1) * 512],
                    start=True,
                    stop=True,
                )
            for g in range(4):  # Q (this core's rows = xt cols 0..2047) -> R
                nc.tensor.matmul(
                    R[:, g * 512 : (g + 1) * 512],
                    wq_sb,
                    xt_sb[:, g * 512 : (g + 1) * 512],
                    start=True,
                    stop=True,
                )
            for g in range(2):
                nc.vector.tensor_scalar_add(
                    out=kt_sb[:, g * 1024 : (g + 1) * 1024],
                    in0=L[:, g * 1024 : (g + 1) * 1024],
                    scalar1=bks,
                )
            nc.vector.tensor_scalar_add(out=qt_sb[:, 0:128], in0=R[:, 0:128], scalar1=bqs)
            nc.vector.tensor_scalar_add(out=qt_sb[:, 128:2048], in0=R[:, 128:2048], scalar1=bqs)
            for g in range(4):  # K right -> R (after qt drained)
                nc.tensor.matmul(
                    R[:, g * 512 : (g + 1) * 512],
                    wk_sb,
                    xt_sb[:, 2048 + g * 512 : 2048 + (g + 1) * 512],
                    start=True,
                    stop=True,
                )
            for g in range(2):
                nc.vector.tensor_scalar_add(
                    out=kt_sb[:, 2048 + g * 1024 : 2048 + (g + 1) * 1024],
                    in0=R[:, g * 1024 : (g + 1) * 1024],
                    scalar1=bks,
                )

            E_tiles = {}

            def emit_S(i, half):
                reg = L if half == 0 else R
                lhsT = qt_sb[:, i * 128 : (i + 1) * 128]
                for g in (1, 2, 3, 0):
                    nc.tensor.matmul(
                        reg[:, g * 512 : (g + 1) * 512],
                        lhsT,
                        kt_sb[:, half * 2048 + g * 512 : half * 2048 + (g + 1) * 512],
                        start=True,
                        stop=True,
                    )

            def emit_exp(i, half):
                if i not in E_tiles:
                    E_tiles[i] = epool.tile([128, N], BF16, tag="E", name=f"E_{i}")
                reg = L if half == 0 else R
                nc.scalar.activation(
                    out=E_tiles[i][:, half * 2048 : (half + 1) * 2048],
                    in_=reg,
                    func=AF.Exp,
                    accum_out=part[:, 2 * i + half : 2 * i + half + 1],
                )

            def emit_rr(i):
                nc.vector.tensor_add(
                    out=Rcol[:, i : i + 1],
                    in0=part[:, 2 * i : 2 * i + 1],
                    in1=part[:, 2 * i + 1 : 2 * i + 2],
                )
                nc.vector.reciprocal(out=rr[:, i : i + 1], in_=Rcol[:, i : i + 1])
                p = i % 2
                for j in range(4):
                    nc.gpsimd.tensor_copy(
                        out=rmat[:, p, j, 32 * j : 32 * j + 1], in_=rr[:, i : i + 1]
                    )

            def emit_w(i, half):
                """Rank-1 contraction of E tile i, m-half `half`, into bank 0
                of L (half 0) or R (half 1); chunk j lands on partition 32j.
                Then CAST (not RMW) to a bf16 slot; GPSIMD accumulates."""
                reg = L if half == 0 else R
                p = i % 2
                E = E_tiles[i]
                for j in range(4):
                    m0 = half * 2048 + j * 512
                    nc.tensor.matmul(
                        reg[:, 0:512],
                        rmat[:, p, j, :],
                        E[:, m0 : m0 + 512],
                        start=(j == 0),
                        stop=(j == 3),
                        skip_group_check=True,
                    )
                slot = (2 * i + half) % 4
                nc.vector.tensor_copy(out=wstore[:, slot, :], in_=reg[:, 0:512])
                nc.gpsimd.tensor_add(
                    out=wacc[:, half * 512 : (half + 1) * 512],
                    in0=wacc[:, half * 512 : (half + 1) * 512],
                    in1=wstore[:, slot, :],
                )

            # ---- prologue: first tile ----
            emit_S(0, 0)
            emit_exp(0, 0)
            emit_S(0, 1)
            emit_exp(0, 1)

            # ---- main loop ----
            # Steady state: during exp(i,1) PE runs w(i-1,0)+S(i+1,0); during
            # exp(i+1,0) PE runs w(i-1,1)+S(i+1,1).  The L-chain after
            # exp_L(i) is w(4 MM) -> cast(DVE) -> S(2 MM) -> exp_L(i+1).
            for i in range(RT):
                emit_rr(i)
                if i + 1 < RT:
                    emit_S(i + 1, 0)
                    emit_exp(i + 1, 0)
                if i >= 1:
                    emit_w(i - 1, 1)
                if i + 1 < RT:
                    emit_S(i + 1, 1)
                    emit_exp(i + 1, 1)
                emit_w(i, 0)
            emit_w(RT - 1, 1)

            # ---- epilogue: V fused; out[d] = (1/N) sum_m w[m] V0[m,d] ----
            for c in range(4):  # 1024-col m chunks
                hf, jb = c // 2, 2 * (c % 2)
                vreg = L[:, (c % 2) * 1024 : (c % 2 + 1) * 1024]
                for g in range(2):  # V chunk from xt (no bias; host adds bv)
                    nc.tensor.matmul(
                        vreg[:, g * 512 : (g + 1) * 512],
                        wv_sb,
                        xt_sb[:, c * 1024 + g * 512 : c * 1024 + (g + 1) * 512],
                        start=True,
                        stop=True,
                    )
                for jj in (jb, jb + 1):
                    nc.tensor.matmul(  # replicate w segment to all partitions
                        R[:, (c % 2) * 1024 + (jj - jb) * 512 : (c % 2) * 1024 + (jj - jb + 1) * 512],
                        ones_sb[32 * jj : 32 * jj + 1, :],
                        wacc[32 * jj : 32 * jj + 1, hf * 512 : (hf + 1) * 512],
                        start=True,
                        stop=True,
                        tile_position=(32 * jj, 0),
                    )
                scr = escr[:, (c % 2) * 1024 : (c % 2 + 1) * 1024]
                vch = vsb[:, (c % 2) * 1024 : (c % 2 + 1) * 1024]
                # DVE can read only one PSUM operand: stage V in SBUF first
                nc.vector.tensor_copy(out=vch, in_=vreg)
                nc.vector.tensor_mul(
                    out=scr, in0=vch, in1=R[:, (c % 2) * 1024 : (c % 2 + 1) * 1024]
                )
                if c % 2 == 0:
                    nc.scalar.activation(
                        out=odump, in_=scr, func=AF.Identity, accum_out=opart[:, c : c + 1]
                    )
                else:
                    nc.vector.tensor_reduce(
                        out=opart[:, c : c + 1], in_=scr, axis=mybir.AxisListType.X, op=ALU.add
                    )
            nc.vector.tensor_reduce(out=o1, in_=opart, axis=mybir.AxisListType.X, op=ALU.add)
            nc.scalar.activation(out=o128, in_=o1, func=AF.Identity, scale=1.0 / N)
            nc.sync.dma_start(out[:, :], o128)

    nc.compile()
    return nc


_cache = {}


def get_nc():
    if "nc" not in _cache:
        _cache["nc"] = build_nc()
    return _cache["nc"]


def make_in_maps(x, Wq, bq, Wk, bk, Wv, bv):
    x = np.asarray(x, np.float32)
    wqT = np.ascontiguousarray(np.asarray(Wq, np.float32).T.astype(NPBF))
    wkT = np.ascontiguousarray(np.asarray(Wk, np.float32).T.astype(NPBF))
    wvT = np.ascontiguousarray(np.asarray(Wv, np.float32).T.astype(NPBF))
    bqc = np.ascontiguousarray(np.asarray(bq, np.float32).reshape(D, 1))
    bkc = np.ascontiguousarray(np.asarray(bk, np.float32).reshape(D, 1))
    in_maps = []
    for c in range(NCORES):
        b = c // 2
        h = c % 2
        xbT = x[b].T.astype(NPBF)  # [128, 4096]
        xperm = np.concatenate(
            [xbT[:, h * HALF : (h + 1) * HALF], xbT[:, (1 - h) * HALF : (2 - h) * HALF]], axis=1
        )
        # contiguous 512-col DMA pieces: [8, 128, 512]
        xp = np.ascontiguousarray(xperm.reshape(D, 8, 512).transpose(1, 0, 2))
        in_maps.append(
            {"xt": xp, "wqT": wqT, "wkT": wkT, "wvT": wvT, "bq": bqc, "bk": bkc}
        )
    return in_maps


def combine(results, bv):
    outs = [np.asarray(results[c]["out"]).reshape(D) for c in range(NCORES)]
    bvf = np.asarray(bv, np.float32).reshape(D)
    return np.stack([outs[2 * b] + outs[2 * b + 1] + bvf for b in range(B)]).astype(np.float32)


def run(inputs, trace=False, **kwargs):
    from concourse.bass_utils import run_bass_kernel_spmd

    nc = get_nc()
    in_maps = make_in_maps(**inputs)
    res = run_bass_kernel_spmd(nc, in_maps, core_ids=list(range(NCORES)), trace=trace, **kwargs)
    return combine(res.results, inputs["bv"]), res


def kernel(x, Wq, bq, Wk, bk, Wv, bv):
    out, _ = run(dict(x=x, Wq=Wq, bq=bq, Wk=Wk, bk=bk, Wv=Wv, bv=bv))
    return out


# revision 12
# speedup vs baseline: 1.0736x; 1.0290x over previous
"""AttentionAggregation kernel for 8 TRN2 NeuronCores (v3).

Math: out[b] = mean_n softmax(Q K^T)[n,:] @ V  with Q/K/V = x @ W^T + b.
Fold: out[b,d] = sum_m w[b,m] V[b,m,d],  w[b,m] = (1/N) sum_n exp(S[n,m])/R[n],
R[n] = sum_m exp(S[n,m]).  attn@V collapses to rank-1 matmuls (r^T @ E) plus a
single weighted reduction against V.  Softmax max-subtraction skipped (|S|<~25).

Sharding: core c -> batch b=c//2, softmax-row half h=c%2 (2048 rows each).
Host permutes x[b].T columns so each core's own row-half comes first (the m
axis is consistently permuted for K/V; softmax and the final sum are
permutation-invariant).  Host sums the two per-core partials and adds bv
(exact: each core's sum_m w[m] = 0.5).

v3 pipeline notes (trace-driven):
- ACT is the pacing engine: one 2048-wide exp = (2048+352)/1.2 = 2.0us, two
  per 128-row tile + 2 READ_ACC = 4.57us/tile floor.
- PSUM = two persistent [128,2048] tiles L (banks 0-3) / R (banks 4-7).
  Tile-framework deps are PSUM-tile-granular, so everything touching L
  serializes: exp_L(i) -> w(i-1,0) -> cast -> S(i+1,0) -> exp_L(i+1).
  v3 shortens that chain: the w psum is CAST (not read-modify-added) to a
  bf16 slot on DVE, and the accumulation into wacc happens on the otherwise
  idle GPSIMD engine, off the chain.  rmat copies also live on GPSIMD.
- V projection is fused into the epilogue (V chunks computed from xt right
  before the final multiply), so the main loop/prologue carry no V work.
- x arrives as 8 contiguous [128,512] pieces (1KB DMA elements instead of
  512B strided rows) issued from sync+scalar+gpsimd queues in parallel.
- PE HAM warm-up dummies + early exp-table load run during the DMA wait.

HW notes (inherited):
- everything PE-facing is bf16 (fp32 matmuls lower to HI/LO pairs).
- no DVE/ACT writes to PSUM banks that matmuls later accumulate into.
- tensor_tensor_reduce faults on HW; keep mult and reduce separate.
"""

import sys

sys.path.insert(0, "/opt/trn_rl_repo")

import ml_dtypes
import numpy as np

import concourse.bass as bass
import concourse.mybir as mybir
import concourse.tile as tile
from concourse import bacc

D = 128
N = 4096
B = 4
NCORES = 8
HALF = N // 2
RT = HALF // 128  # 16 row tiles per core

F32 = mybir.dt.float32
BF16 = mybir.dt.bfloat16
NPBF = ml_dtypes.bfloat16
AF = mybir.ActivationFunctionType
ALU = mybir.AluOpType


def build_nc():
    nc = bacc.Bacc()
    xt = nc.dram_tensor("xt", [8, D, 512], BF16, kind="ExternalInput")  # x[b].T pieces
    wqT = nc.dram_tensor("wqT", [D, D], BF16, kind="ExternalInput")
    wkT = nc.dram_tensor("wkT", [D, D], BF16, kind="ExternalInput")
    wvT = nc.dram_tensor("wvT", [D, D], BF16, kind="ExternalInput")
    bq = nc.dram_tensor("bq", [D, 1], F32, kind="ExternalInput")
    bk = nc.dram_tensor("bk", [D, 1], F32, kind="ExternalInput")
    out = nc.dram_tensor("out", [4, 32], F32, kind="ExternalOutput")

    with tile.TileContext(nc) as tc:
        with (
            tc.tile_pool(name="singles", bufs=1) as singles,
            tc.tile_pool(name="pp", bufs=1, space="PSUM") as pp,
            tc.tile_pool(name="epool", bufs=3) as epool,
        ):
            L = pp.tile([128, 2048], F32, tag="L", name="L")
            R = pp.tile([128, 2048], F32, tag="R", name="R")

            wq_sb = singles.tile([D, D], BF16, tag="wq", name="wq_sb")
            wk_sb = singles.tile([D, D], BF16, tag="wk", name="wk_sb")
            wv_sb = singles.tile([D, D], BF16, tag="wv", name="wv_sb")
            bqs = singles.tile([D, 1], F32, tag="bq", name="bqs")
            bks = singles.tile([D, 1], F32, tag="bk", name="bks")
            ones_sb = singles.tile([D, D], BF16, tag="ones", name="ones_sb")
            tl_out = singles.tile([D, 1], F32, tag="tl", name="tl_out")
            xt_sb = singles.tile([D, N], BF16, tag="xt", name="xt_sb")
            kt_sb = singles.tile([D, N], BF16, tag="kt", name="kt_sb")
            qt_sb = singles.tile([D, HALF], BF16, tag="qt", name="qt_sb")
            part = singles.tile([128, 2 * RT], F32, tag="part", name="part")
            Rcol = singles.tile([128, RT], F32, tag="R", name="Rcol")
            rr = singles.tile([128, RT], F32, tag="rr", name="rr")
            rmat = singles.tile([128, 2, 4, D], BF16, tag="rmat", name="rmat")
            # per-tile w slots (bf16) cast off PSUM; GPSIMD folds them into wacc
            wstore = singles.tile([128, 4, 512], BF16, tag="wst", name="wstore")
            wacc = singles.tile([128, 1024], BF16, tag="wacc", name="wacc")
            escr = singles.tile([128, 2048], F32, tag="escr", name="escr")
            vsb = singles.tile([128, 2048], BF16, tag="vsb", name="vsb")
            odump = singles.tile([128, 1024], F32, tag="odump", name="odump")
            opart = singles.tile([128, 4], F32, tag="opart", name="opart")
            o1 = singles.tile([128, 1], F32, tag="o1", name="o1")
            o128 = singles.tile([128, 32], F32, tag="o128", name="o128")
            o4x32 = singles.tile([128, 32], F32, tag="o4x32", name="o4x32")

            nc.vector.memset(ones_sb, 1.0)
            nc.vector.memset(o128, 0.0)
            nc.vector.memset(rmat, 0.0)
            nc.gpsimd.memset(wacc, 0.0)

            # ---- DMAs: parallel issue across sync / scalar / gpsimd ----
            nc.sync.dma_start(wk_sb, wkT[:, :])
            for c in (0, 2, 4, 6):
                nc.sync.dma_start(xt_sb[:, c * 512 : (c + 1) * 512], xt[c, :, :])
            for c in (1, 3, 5, 7):
                nc.gpsimd.dma_start(xt_sb[:, c * 512 : (c + 1) * 512], xt[c, :, :])
            nc.scalar.dma_start(wq_sb, wqT[:, :])
            nc.scalar.dma_start(bks, bk[:, :])
            nc.scalar.dma_start(bqs, bq[:, :])
            nc.scalar.dma_start(wv_sb, wvT[:, :])

            # early exp table load (~2.7us) while DMAs land
            nc.scalar.activation(out=tl_out, in_=ones_sb[:, 0:1], func=AF.Exp)

            # PE HAM warm-up (keeps clock at 2.4GHz through the prologue)
            for _ in range(36):
                nc.tensor.matmul(R[:, 1024:1152], ones_sb, ones_sb, start=True, stop=True)

            # ---- projections (1024-wide matmuls, bias-add drains on DVE) ----
            for g in range(4):  # K left -> L
                nc.tensor.matmul(
                    L[:, g * 512 : (g + 1) * 512],
                    wk_sb,
                    xt_sb[:, g * 512 : (g + 1) * 512],
                    start=True,
                    stop=True,
                )
            for g in range(4):  # Q (this core's rows = xt cols 0..2047) -> R
                nc.tensor.matmul(
                    R[:, g * 512 : (g + 1) * 512],
                    wq_sb,
                    xt_sb[:, g * 512 : (g + 1) * 512],
                    start=True,
                    stop=True,
                )
            for g in range(2):
                nc.vector.tensor_scalar_add(
                    out=kt_sb[:, g * 1024 : (g + 1) * 1024],
                    in0=L[:, g * 1024 : (g + 1) * 1024],
                    scalar1=bks,
                )
            nc.vector.tensor_scalar_add(out=qt_sb[:, 0:128], in0=R[:, 0:128], scalar1=bqs)
            nc.vector.tensor_scalar_add(out=qt_sb[:, 128:2048], in0=R[:, 128:2048], scalar1=bqs)
            for g in range(4):  # K right -> R (after qt drained)
                nc.tensor.matmul(
                    R[:, g * 512 : (g + 1) * 512],
                    wk_sb,
                    xt_sb[:, 2048 + g * 512 : 2048 + (g + 1) * 512],
                    start=True,
                    stop=True,
                )
            for g in range(2):
                nc.vector.tensor_scalar_add(
                    out=kt_sb[:, 2048 + g * 1024 : 2048 + (g + 1) * 1024],
                    in0=R[:, g * 1024 : (g + 1) * 1024],
                    scalar1=bks,
                )

            E_tiles = {}

            def emit_S(i, half):
                # g1-g3 first (their only dep is the previous exp read); g0 is
                # emitted as a separate batch so the w-cast wait lands on it
                # alone and banks 1-3 stage concurrently with the cast.
                reg = L if half == 0 else R
                lhsT = qt_sb[:, i * 128 : (i + 1) * 128]
                for g in (1, 2, 3):
                    nc.tensor.matmul(
                        reg[:, g * 512 : (g + 1) * 512],
                        lhsT,
                        kt_sb[:, half * 2048 + g * 512 : half * 2048 + (g + 1) * 512],
                        start=True,
                        stop=True,
                    )
                nc.tensor.matmul(
                    reg[:, 0:512],
                    lhsT,
                    kt_sb[:, half * 2048 : half * 2048 + 512],
                    start=True,
                    stop=True,
                )

            def emit_exp(i, half):
                if i not in E_tiles:
                    E_tiles[i] = epool.tile([128, N], BF16, tag="E", name=f"E_{i}")
                reg = L if half == 0 else R
                nc.scalar.activation(
                    out=E_tiles[i][:, half * 2048 : (half + 1) * 2048],
                    in_=reg,
                    func=AF.Exp,
                    accum_out=part[:, 2 * i + half : 2 * i + half + 1],
                )

            def emit_rr(i):
                nc.vector.tensor_add(
                    out=Rcol[:, i : i + 1],
                    in0=part[:, 2 * i : 2 * i + 1],
                    in1=part[:, 2 * i + 1 : 2 * i + 2],
                )
                nc.vector.reciprocal(out=rr[:, i : i + 1], in_=Rcol[:, i : i + 1])
                p = i % 2
                for j in range(4):
                    nc.gpsimd.tensor_copy(
                        out=rmat[:, p, j, 32 * j : 32 * j + 1], in_=rr[:, i : i + 1]
                    )

            def emit_w(i, half, home=0, accum_dve=False):
                """Rank-1 contraction of E tile i, m-half `half`, into the
                512-col bank at `home` of L (half 0) / R (half 1); chunk j
                lands on partition 32j.  CAST (not RMW) to a bf16 slot on
                DVE; the wacc accumulate runs on GPSIMD (or DVE in the tail
                so the epilogue is not gated on the slow GPSIMD queue)."""
                reg = L if half == 0 else R
                p = i % 2
                E = E_tiles[i]
                for j in range(4):
                    m0 = half * 2048 + j * 512
                    nc.tensor.matmul(
                        reg[:, home : home + 512],
                        rmat[:, p, j, :],
                        E[:, m0 : m0 + 512],
                        start=(j == 0),
                        stop=(j == 3),
                        skip_group_check=True,
                    )
                slot = (2 * i + half) % 4
                nc.vector.tensor_copy(out=wstore[:, slot, :], in_=reg[:, home : home + 512])
                eng = nc.vector if accum_dve else nc.gpsimd
                eng.tensor_add(
                    out=wacc[:, half * 512 : (half + 1) * 512],
                    in0=wacc[:, half * 512 : (half + 1) * 512],
                    in1=wstore[:, slot, :],
                )

            # ---- prologue: first tile ----
            emit_S(0, 0)
            emit_exp(0, 0)
            emit_S(0, 1)
            emit_exp(0, 1)

            # ---- main loop ----
            # Steady state: during exp(i,1) PE runs w(i-1,0)+S(i+1,0); during
            # exp(i+1,0) PE runs w(i-1,1)+S(i+1,1).  The L-chain after
            # exp_L(i) is w(4 MM) -> cast(DVE) -> S(2 MM) -> exp_L(i+1).
            for i in range(RT - 1):
                emit_rr(i)
                if i + 1 < RT:
                    emit_S(i + 1, 0)
                    emit_exp(i + 1, 0)
                if i >= 1:
                    emit_w(i - 1, 1)
                if i + 1 < RT:
                    emit_S(i + 1, 1)
                    emit_exp(i + 1, 1)
                emit_w(i, 0)
            # tail: w(15,0) first (its exp finished earlier), distinct psum
            # homes so the three groups don't serialize, accumulate on DVE
            emit_rr(RT - 1)
            emit_w(RT - 1, 0, home=0, accum_dve=True)
            emit_w(RT - 2, 1, home=0, accum_dve=True)
            emit_w(RT - 1, 1, home=512, accum_dve=True)

            # ---- epilogue: V fused; out[d] = (1/N) sum_m w[m] V0[m,d] ----
            for c in range(4):  # 1024-col m chunks
                hf, jb = c // 2, 2 * (c % 2)
                vreg = L[:, (c % 2) * 1024 : (c % 2 + 1) * 1024]
                for g in range(2):  # V chunk from xt (no bias; host adds bv)
                    nc.tensor.matmul(
                        vreg[:, g * 512 : (g + 1) * 512],
                        wv_sb,
                        xt_sb[:, c * 1024 + g * 512 : c * 1024 + (g + 1) * 512],
                        start=True,
                        stop=True,
                    )
                for jj in (jb, jb + 1):
                    nc.tensor.matmul(  # replicate w segment to all partitions
                        R[:, (c % 2) * 1024 + (jj - jb) * 512 : (c % 2) * 1024 + (jj - jb + 1) * 512],
                        ones_sb[32 * jj : 32 * jj + 1, :],
                        wacc[32 * jj : 32 * jj + 1, hf * 512 : (hf + 1) * 512],
                        start=True,
                        stop=True,
                        tile_position=(32 * jj, 0),
                    )
                scr = escr[:, (c % 2) * 1024 : (c % 2 + 1) * 1024]
                vch = vsb[:, (c % 2) * 1024 : (c % 2 + 1) * 1024]
                # DVE can read only one PSUM operand: stage V in SBUF first
                nc.vector.tensor_copy(out=vch, in_=vreg)
                nc.vector.tensor_mul(
                    out=scr, in0=vch, in1=R[:, (c % 2) * 1024 : (c % 2 + 1) * 1024]
                )
                if c % 2 == 0:
                    nc.scalar.activation(
                        out=odump, in_=scr, func=AF.Identity, accum_out=opart[:, c : c + 1]
                    )
                else:
                    nc.vector.tensor_reduce(
                        out=opart[:, c : c + 1], in_=scr, axis=mybir.AxisListType.X, op=ALU.add
                    )
            nc.vector.tensor_reduce(out=o1, in_=opart, axis=mybir.AxisListType.X, op=ALU.add)
            nc.scalar.activation(out=o128[:, 0:1], in_=o1, func=AF.Identity, scale=1.0 / N)
            # pack [128,1] -> rows {0,32,64,96} x 32 cols via 32x32 block
            # transpose so the output DMA moves 4 contiguous 128B rows
            # instead of 128 strided 4B elements (saves ~7us of DMA/teardown)
            nc.vector.transpose(out=o4x32, in_=o128)
            for b in range(4):
                nc.sync.dma_start(out[b : b + 1, :], o4x32[32 * b : 32 * b + 1, :])

    nc.compile()
    return nc


_cache = {}


def get_nc():
    if "nc" not in _cache:
        _cache["nc"] = build_nc()
    return _cache["nc"]


def make_in_maps(x, Wq, bq, Wk, bk, Wv, bv):
    x = np.asarray(x, np.float32)
    wqT = np.ascontiguousarray(np.asarray(Wq, np.float32).T.astype(NPBF))
    wkT = np.ascontiguousarray(np.asarray(Wk, np.float32).T.astype(NPBF))
    wvT = np.ascontiguousarray(np.asarray(Wv, np.float32).T.astype(NPBF))
    bqc = np.ascontiguousarray(np.asarray(bq, np.float32).reshape(D, 1))
    bkc = np.ascontiguousarray(np.asarray(bk, np.float32).reshape(D, 1))
    in_maps = []
    for c in range(NCORES):
        b = c // 2
        h = c % 2
        xbT = x[b].T.astype(NPBF)  # [128, 4096]
        xperm = np.concatenate(
            [xbT[:, h * HALF : (h + 1) * HALF], xbT[:, (1 - h) * HALF : (2 - h) * HALF]], axis=1
        )
        # contiguous 512-col DMA pieces: [8, 128, 512]
        xp = np.ascontiguousarray(xperm.reshape(D, 8, 512).transpose(1, 0, 2))
        in_maps.append(
            {"xt": xp, "wqT": wqT, "wkT": wkT, "wvT": wvT, "bq": bqc, "bk": bkc}
        )
    return in_maps


def combine(results, bv):
    outs = [np.asarray(results[c]["out"]).reshape(D) for c in range(NCORES)]
    bvf = np.asarray(bv, np.float32).reshape(D)
    return np.stack([outs[2 * b] + outs[2 * b + 1] + bvf for b in range(B)]).astype(np.float32)


def run(inputs, trace=False, **kwargs):
    from concourse.bass_utils import run_bass_kernel_spmd

    nc = get_nc()
    in_maps = make_in_maps(**inputs)
    res = run_bass_kernel_spmd(nc, in_maps, core_ids=list(range(NCORES)), trace=trace, **kwargs)
    return combine(res.results, inputs["bv"]), res


def kernel(x, Wq, bq, Wk, bk, Wv, bv):
    out, _ = run(dict(x=x, Wq=Wq, bq=bq, Wk=Wk, bk=bk, Wv=Wv, bv=bv))
    return out
